# revision 16
# baseline (speedup 1.0000x reference)
"""Chamfer distance loss on 8 Trainium2 NeuronCores.

Full inputs: points1 [16, 4096, 3], points2 [16, 4096, 3] (fp32).
Output: scalar fp32 loss = (sum(min_m dist) + sum(min_n dist)) / B.

Sharding: data-parallel over batch B=16 -> 2 batches per core on 8 cores.
Each core computes a partial scalar (sum of row-mins + col-mins for its
batches); host sums the 8 partials and divides by B.

Per-batch device algorithm (per core):
  dist[n, m] = |a_n|^2 + |b_m|^2 - 2 a.b  computed as:
    psum = matmul(lhsT=[ax,ay,az,-.5,-.5,-.5], rhs=[bx,by,bz,bx^2,by^2,bz^2])
         = a.b - |b|^2/2                       (K=6, fp32r, N=512 per bank)
    dist = ScalarE Identity((-2)*psum + bias)  bias = |a_n|^2 per partition
  row-min: DVE reduce-min over free dim, col-min: DVE tensor_tensor min
  accumulated across row stripes; final col-min across partitions via
  PE transpose + reduce-min; sums via reduce-add + matmul with ones.
"""

import time

import numpy as np

import concourse.bacc as bacc
import concourse.mybir as mybir
import concourse.tile as tile
from concourse import bass_utils
from concourse.masks import make_identity

N_CORES = 8

f32 = mybir.dt.float32
f32r = mybir.dt.float32r
f16 = mybir.dt.bfloat16
AF = mybir.ActivationFunctionType
ALU = mybir.AluOpType
AX = mybir.AxisListType

_CACHE = {}
last_exec_seconds = None  # wall time of the device dispatch (set per call)

USE_TTR = False  # fused tensor_tensor_reduce row-min (all-zero/err on HW)
USE_T16 = True   # 16-bit PE transposes for the col-min fold


def _build(bl: int, n: int, m: int):
    """Build the SPMD module for bl batches of [n x 3] vs [m x 3] points."""
    assert n % 128 == 0 and m % 2048 == 0
    n_stripes = n // 128
    n_groups = m // 2048
    assert n_groups <= 2, "row-min TTR fusion covers at most 2 groups"

    nc = bacc.Bacc("TRN2", target_bir_lowering=False, debug=False)
    p1 = nc.dram_tensor("p1T", [bl, 3, n], f32, kind="ExternalInput")
    p2 = nc.dram_tensor("p2T", [bl, 3, m], f32, kind="ExternalInput")
    out = nc.dram_tensor("out", [1, bl], f32, kind="ExternalOutput")

    with tile.TileContext(nc) as tc:
        with (
            tc.tile_pool(name="const", bufs=1) as constp,
            tc.tile_pool(name="pts", bufs=2) as ptsp,
            tc.tile_pool(name="sq", bufs=1) as sqp,
            tc.tile_pool(name="acc", bufs=2) as accp,
            tc.tile_pool(name="dist", bufs=4) as distp,
            tc.tile_pool(name="small", bufs=4) as smallp,
            tc.tile_pool(name="psum", bufs=2, space="PSUM") as psump,
        ):
            ident = constp.tile([128, 128], f16 if USE_T16 else f32)
            make_identity(nc, ident[:])
            ones128 = constp.tile([128, 1], f32)
            nc.gpsimd.memset(ones128[:], 1.0)
            ones3 = constp.tile([3, 1], f32)
            nc.gpsimd.memset(ones3[:], 1.0)
            out_sb = constp.tile([1, bl], f32)

            for b in range(bl):
                a6 = ptsp.tile([6, n], f32r, tag="a6")
                b6 = ptsp.tile([6, m], f32r, tag="b6")
                asq = sqp.tile([3, n], f32, tag="asq")
                bsq = sqp.tile([3, m], f32, tag="bsq")

                stage_a = sqp.tile([6, n], f32, tag="stage")
                nc.gpsimd.memset(stage_a[:], -0.5)
                nc.sync.dma_start(stage_a[0:3, :], p1.ap()[b])
                nc.gpsimd.tensor_tensor(
                    asq[:], stage_a[0:3, :], stage_a[0:3, :], ALU.mult
                )
                nc.scalar.copy(a6[:], stage_a[:])

                stage_b = sqp.tile([6, m], f32, tag="stage")
                nc.sync.dma_start(stage_b[0:3, :], p2.ap()[b])
                nc.gpsimd.tensor_tensor(
                    bsq[:], stage_b[0:3, :], stage_b[0:3, :], ALU.mult
                )
                nc.sync.dma_start(stage_b[3:6, :], bsq[:])
                nc.scalar.copy(b6[:], stage_b[:])

                # |a_n|^2 as per-partition bias columns: a2c[p, s] for stripe s
                a2psum = psump.tile([128, 2048], f32, tag="mm")
                for t in range(n_stripes):
                    nc.tensor.matmul(
                        a2psum[:, t : t + 1],
                        asq[:, 128 * t : 128 * (t + 1)],
                        ones3[:],
                        start=True,
                        stop=True,
                    )
                a2c = smallp.tile([128, n_stripes], f32, tag="a2c")
                nc.vector.tensor_copy(a2c[:], a2psum[:, 0:n_stripes])

                acc = accp.tile([128, m], f16, tag="acc")
                rowmin = smallp.tile([128, n_stripes], f16, tag="rowmin")
                for s in range(n_stripes):
                    lhsT = a6[:, 128 * s : 128 * (s + 1)]
                    dts = []
                    for g in range(n_groups):
                        ps = psump.tile([128, 2048], f32, tag="mm")
                        for j in range(4):
                            mo = 2048 * g + 512 * j
                            nc.tensor.matmul(
                                ps[:, 512 * j : 512 * (j + 1)],
                                lhsT,
                                b6[:, mo : mo + 512],
                                start=True,
                                stop=True,
                            )
                        dt_ = distp.tile([128, 2048], f16, tag=f"dist{g}")
                        nc.scalar.activation(
                            dt_[:], ps[:], AF.Identity,
                            bias=a2c[:, s : s + 1], scale=-2.0,
                        )
                        gsl = slice(2048 * g, 2048 * (g + 1))
                        if s == 0:
                            nc.vector.tensor_copy(acc[:, gsl], dt_[:])
                        else:
                            nc.vector.tensor_tensor(
                                acc[:, gsl], acc[:, gsl], dt_[:], ALU.min
                            )
                        dts.append(dt_)
                    # fused row-min for the stripe: elementwise min of the two
                    # 2048-wide groups + min-reduce, one DVE instruction.
                    if USE_TTR:
                        t01 = distp.tile([128, 2048], f16, tag="t01")
                        in1 = dts[1] if n_groups >= 2 else dts[0]
                        nc.vector.tensor_tensor_reduce(
                            out=t01[:],
                            in0=dts[0][:],
                            in1=in1[:],
                            scale=1.0,
                            scalar=3.0e38,
                            op0=ALU.min,
                            op1=ALU.min,
                            accum_out=rowmin[:, s : s + 1],
                        )
                    else:
                        t01 = distp.tile([128, 2048], f16, tag="t01")
                        if n_groups >= 2:
                            nc.vector.tensor_tensor(
                                t01[:], dts[0][:], dts[1][:], ALU.min
                            )
                        else:
                            nc.vector.tensor_copy(t01[:], dts[0][:])
                        nc.vector.tensor_tensor(
                            t01[:, 0:1024], t01[:, 0:1024], t01[:, 1024:2048],
                            ALU.min,
                        )
                        nc.vector.tensor_tensor(
                            t01[:, 0:512], t01[:, 0:512], t01[:, 512:1024],
                            ALU.min,
                        )
                        nc.vector.tensor_reduce(
                            rowmin[:, s : s + 1], t01[:, 0:512], axis=AX.X,
                            op=ALU.min,
                        )

                # col-min across partitions: 16 transposes per psum tile,
                # then one strided reduce-min per psum tile.
                n_blocks = m // 128
                if USE_T16:
                    acc_t = acc
                else:
                    acc_t = accp.tile([128, m], f32, tag="acc32")
                    nc.vector.tensor_copy(acc_t[:], acc[:])
                tdt = f16 if USE_T16 else f32
                cmin = smallp.tile([128, n_blocks], tdt, tag="cmin")
                tpb = 16 if USE_T16 else 8  # transposes per psum tile
                for k0 in range(0, n_blocks, tpb):
                    pst = psump.tile([128, 2048 if USE_T16 else 1024], tdt, tag="mm")
                    kk = min(tpb, n_blocks - k0)
                    for k in range(kk):
                        nc.tensor.transpose(
                            pst[:, 128 * k : 128 * (k + 1)],
                            acc_t[:, 128 * (k0 + k) : 128 * (k0 + k + 1)],
                            ident[:],
                        )
                    nc.vector.tensor_reduce(
                        cmin[:, k0 : k0 + kk],
                        pst[:, 0 : 128 * kk].rearrange("p (k x) -> p k x", x=128),
                        axis=AX.X,
                        op=ALU.min,
                    )

                rs = smallp.tile([128, 1], f32, tag="rs")
                cs = smallp.tile([128, 1], f32, tag="cs")
                nc.vector.tensor_reduce(rs[:], rowmin[:], axis=AX.X, op=ALU.add)
                nc.vector.tensor_reduce(cs[:], cmin[:], axis=AX.X, op=ALU.add)
                sc = psump.tile([128, 2048], f32, tag="mm")
                nc.tensor.matmul(sc[0:1, 0:1], rs[:], ones128[:], start=True, stop=False)
                nc.tensor.matmul(sc[0:1, 0:1], cs[:], ones128[:], start=False, stop=True)
                nc.vector.tensor_copy(out_sb[0:1, b : b + 1], sc[0:1, 0:1])

            nc.sync.dma_start(out.ap(), out_sb[:])

    nc.finalize()
    return nc


def kernel(points1, points2):
    global last_exec_seconds
    points1 = np.ascontiguousarray(np.asarray(points1), dtype=np.float32)
    points2 = np.ascontiguousarray(np.asarray(points2), dtype=np.float32)
    btot, n, _ = points1.shape
    m = points2.shape[1]
    bl = btot // N_CORES

    key = (bl, n, m)
    if _CACHE.get("key") != key:
        _CACHE["nc"] = _build(bl, n, m)
        _CACHE["key"] = key
    nc = _CACHE["nc"]

    p1t = np.ascontiguousarray(points1.transpose(0, 2, 1))  # [B, 3, n]
    p2t = np.ascontiguousarray(points2.transpose(0, 2, 1))  # [B, 3, m]
    in_maps = [
        {
            "p1T": p1t[c * bl : (c + 1) * bl],
            "p2T": p2t[c * bl : (c + 1) * bl],
        }
        for c in range(N_CORES)
    ]
    t0 = time.time()
    res = bass_utils.run_bass_kernel_spmd(
        nc, in_maps, core_ids=list(range(N_CORES))
    )
    last_exec_seconds = time.time() - t0

    total = np.float64(0.0)
    for r in res.results:
        total += r["out"].astype(np.float64).sum()
    return np.float32(total / btot)
